# revision 1
# baseline (speedup 1.0000x reference)
"""Trainium2 Bass kernel for a 2-layer GRU network + sigmoid classifier.

Reference computation (PyTorch-style GRU, gate order r,z,n):
    h1 = GRU0(x);  h2 = GRU1(h1);  out = sigmoid(h2[24] @ W_cls.T + b_cls)

Only h2[24] is consumed, so only timesteps 0..24 of both layers are needed.

Strategy (8 NeuronCores, data-parallel over batch: 512 -> 64 per core):
  - Layout: gate/hidden dim on SBUF partitions, batch on the free dim.
    This avoids every transpose in the recurrence and lets biases fold
    into per-partition ACT bias during PSUM->SBUF copyback.
  - Phase 1: xg0 = W_ih0 @ x + (b_ih0+b_hh0) for all 25 steps as one big
    fp32 matmul (weights stationary, x columns moving), spilled to DRAM.
  - Phase 2: layer-0 scan, 25 steps. Recurrent matmul in fp16 (weights
    stationary [128x128] tiles w/ fast-weight-load, h moving, fp32 PSUM).
    Gate math: DVE adds/muls + ScalarE sigmoid/tanh.
  - Phase 3: xg1 = W_ih1 @ h1 + biases, batched fp16 matmul.
  - Phase 4: layer-1 scan; at t=24 fused classifier matmul + sigmoid.
"""

import numpy as np

SEQ_USED = 25          # classifier reads h2[24]
BATCH = 512
IN_DIM = 512
HID = 768
NCORES = 8
B = BATCH // NCORES    # 64 per core
N = SEQ_USED * B       # 1600 moving columns in the batched projections
KI = IN_DIM // 128     # 4
KH = HID // 128        # 6
M3 = 3 * HID // 128    # 18 gate row-tiles (r: 0..5, z: 6..11, n: 12..17)
NCH = 4                # batched-projection column chunks
NW = N // NCH          # 400 columns per chunk (<=512 fp32 moving limit)

_CACHE = {}


def _build():
    """Build the SPMD Bass program (identical on all 8 cores)."""
    import concourse.mybir as mybir
    import concourse.tile as tile
    from concourse import bacc

    f32 = mybir.dt.float32
    f16 = mybir.dt.float16
    AF = mybir.ActivationFunctionType

    # Bacc (not raw Bass): its compile() legalizes sync waits for TRN2
    # (move_matmul_waits_to_ldweights + generate_event_semaphores), without
    # which walrus rejects any instruction carrying >1 semaphore wait.
    nc = bacc.Bacc("TRN2", target_bir_lowering=False, debug=False)

    # ---- I/O ----
    xT_d = nc.dram_tensor("xT", [128, KI, N], f16, kind="ExternalInput")
    wih0_d = nc.dram_tensor("wih0", [128, KI, 3 * HID], f16, kind="ExternalInput")
    whh0_d = nc.dram_tensor("whh0", [128, KH, 3 * HID], f16, kind="ExternalInput")
    wih1_d = nc.dram_tensor("wih1", [128, KH, 3 * HID], f16, kind="ExternalInput")
    whh1_d = nc.dram_tensor("whh1", [128, KH, 3 * HID], f16, kind="ExternalInput")
    bias0_d = nc.dram_tensor("bias0", [128, M3], f32, kind="ExternalInput")
    bias1_d = nc.dram_tensor("bias1", [128, M3], f32, kind="ExternalInput")
    wcls_d = nc.dram_tensor("wcls", [128, KH], f16, kind="ExternalInput")
    bcls_d = nc.dram_tensor("bcls64", [B, 1], f32, kind="ExternalInput")
    y_d = nc.dram_tensor("y", [B, 1], f32, kind="ExternalOutput")
    # DRAM scratch for the input-gate projections of the active layer.
    xg_d = nc.dram_tensor("xg_scratch", [128, M3, N], f32, kind="Internal")

    with tile.TileContext(nc) as tc:
        with (
            tc.tile_pool(name="const", bufs=1) as cpool,
            tc.tile_pool(name="work", bufs=3) as work,
            tc.tile_pool(name="xgio", bufs=4) as xgio,
        ):
            # ---- resident constants ----
            # DMA order: phase-1 inputs first (they gate the first matmuls);
            # scan/phase-3 weights afterwards (not needed until later).
            with tc.tile_pool(name="ph1", bufs=1) as ph1:
                xT_sb = ph1.tile([128, KI, N], f16)
                nc.sync.dma_start(xT_sb, xT_d.ap())
                wih0_sb = ph1.tile([128, KI, 3 * HID], f16)
                nc.sync.dma_start(wih0_sb, wih0_d.ap())
                bias0_sb = cpool.tile([128, M3], f32)
                nc.sync.dma_start(bias0_sb, bias0_d.ap())
                whh0_sb = cpool.tile([128, KH, 3 * HID], f16)
                nc.sync.dma_start(whh0_sb, whh0_d.ap())
                wih1_sb = cpool.tile([128, KH, 3 * HID], f16)
                nc.sync.dma_start(wih1_sb, wih1_d.ap())
                whh1_sb = cpool.tile([128, KH, 3 * HID], f16)
                nc.sync.dma_start(whh1_sb, whh1_d.ap())
                bias1_sb = cpool.tile([128, M3], f32)
                nc.sync.dma_start(bias1_sb, bias1_d.ap())
                wcls_sb = cpool.tile([128, KH], f16)
                nc.sync.dma_start(wcls_sb, wcls_d.ap())
                bcls_sb = cpool.tile([B, 1], f32)
                nc.sync.dma_start(bcls_sb, bcls_d.ap())
                h1T = cpool.tile([128, KH, N], f16)    # layer-0 outputs
                zstate = cpool.tile([128, KH, B], f16)  # h(-1) == 0
                nc.vector.memset(zstate, 0.0)

                # ---- phase 1: xg0 = W_ih0 @ x + bias0 -> DRAM ----
                with tc.tile_pool(name="psA", bufs=2, space="PSUM") as psA:
                    for m in range(M3):
                        for nch in range(NCH):
                            ps = psA.tile([128, NW], f32, tag=f"psA{nch}")
                            for k in range(KI):
                                nc.tensor.matmul(
                                    ps,
                                    wih0_sb[:, k, m * 128:(m + 1) * 128],
                                    xT_sb[:, k, nch * NW:(nch + 1) * NW],
                                    start=(k == 0),
                                    stop=(k == KI - 1),
                                )
                            stage = xgio.tile([128, NW], f32, tag="xgstage")
                            nc.vector.tensor_scalar_add(
                                stage, ps, bias0_sb[:, m:m + 1]
                            )
                            nc.sync.dma_start(
                                xg_d.ap()[:, m, nch * NW:(nch + 1) * NW], stage
                            )

            # ---- scan helper (one GRU step, gate layout [128, KH, B]) ----
            # PE emits gate blocks in order r, n, z so the serial n-gate
            # chain (mul/add/tanh) overlaps the z matmul block; only the
            # z chain (add/sigmoid/mul/add) remains in the per-step tail.
            def gru_step(t, whh_sb, hprev, hnew_out, psum_pool):
                xgt = work.tile([128, M3, B], f32, tag="xgt")
                nc.sync.dma_start(xgt, xg_d.ap()[:, :, t * B:(t + 1) * B])
                pg = {}
                for g in (0, 2, 1):          # r, n, z
                    p = psum_pool.tile([128, KH, B], mybir.dt.float32,
                                       tag=f"pg{g}", name=f"pg{g}")
                    pg[g] = p
                    for i in range(KH):
                        m = g * KH + i
                        for k in range(KH):
                            nc.tensor.matmul(
                                p[:, i, :],
                                whh_sb[:, k, m * 128:(m + 1) * 128],
                                hprev[:, k, :],
                                start=(k == 0),
                                stop=(k == KH - 1),
                            )
                    if g == 0:
                        # r = sigmoid(xg_r + hg_r): runs under the n block
                        rpre = work.tile([128, KH, B], f16, tag="rpre")
                        nc.vector.tensor_add(rpre, pg[0], xgt[:, 0:KH, :])
                        r16 = work.tile([128, KH, B], f16, tag="r16")
                        nc.scalar.activation(r16, rpre, AF.Sigmoid)
                    elif g == 2:
                        # n = tanh(xg_n + r * hg_n): runs under the z block
                        rhn = work.tile([128, KH, B], f32, tag="rhn")
                        nc.vector.tensor_mul(rhn, r16, pg[2])
                        npre = work.tile([128, KH, B], f16, tag="npre")
                        nc.vector.tensor_add(npre, rhn, xgt[:, 2 * KH:3 * KH, :])
                        n16 = work.tile([128, KH, B], f16, tag="n16")
                        nc.scalar.activation(n16, npre, AF.Tanh)
                    else:
                        # zpre first: depends only on the z psum, so DVE can
                        # start it before the tanh-gated d16 below.
                        zpre = work.tile([128, KH, B], f16, tag="zpre")
                        nc.vector.tensor_add(zpre, pg[1], xgt[:, KH:2 * KH, :])
                # tail: z = sigmoid(zpre); h' = n + z * (h - n)
                z16 = work.tile([128, KH, B], f16, tag="z16")
                nc.scalar.activation(z16, zpre, AF.Sigmoid)
                d16 = work.tile([128, KH, B], f16, tag="d16")
                nc.vector.tensor_sub(d16, hprev, n16)
                e16 = work.tile([128, KH, B], f16, tag="e16")
                nc.vector.tensor_mul(e16, z16, d16)
                nc.vector.tensor_add(hnew_out, n16, e16)

            # ---- phases 2-4 fused ----
            # layer-0 scan, with phase-3 (xg1) jobs woven into the PE gaps
            # left by each step's serial tail; then layer-1 scan (its early
            # steps absorb the last xg1 chunk) + classifier at t=24.
            with (
                tc.tile_pool(name="psg", bufs=2, space="PSUM") as psg,
                tc.tile_pool(name="psB", bufs=2, space="PSUM") as psB,
            ):
                def xg1_job(m, nch):
                    ps = psB.tile([128, NW], f32, tag="psB", name="psB")
                    for k in range(KH):
                        nc.tensor.matmul(
                            ps,
                            wih1_sb[:, k, m * 128:(m + 1) * 128],
                            h1T[:, k, nch * NW:(nch + 1) * NW],
                            start=(k == 0),
                            stop=(k == KH - 1),
                        )
                    stage = xgio.tile([128, NW], f32, tag="xgstage")
                    nc.vector.tensor_scalar_add(stage, ps, bias1_sb[:, m:m + 1])
                    nc.sync.dma_start(
                        xg_d.ap()[:, m, nch * NW:(nch + 1) * NW], stage
                    )

                # jobs in nch-major order; chunk nch needs h1 steps
                # < ceil((nch+1)*NW/B), i.e. is ready after scan-0 step
                # {0: 6, 1: 12, 2: 18, 3: 24}.
                jobs = [(m, nch) for nch in range(NCH) for m in range(M3)]
                ready_after = {0: 6, 1: 12, 2: 18, 3: 24}
                ji = 0

                for t in range(SEQ_USED):
                    hprev = zstate if t == 0 else h1T[:, :, (t - 1) * B:t * B]
                    gru_step(t, whh0_sb, hprev,
                             h1T[:, :, t * B:(t + 1) * B], psg)
                    budget = 2
                    while (budget > 0 and ji < len(jobs)
                           and t >= ready_after[jobs[ji][1]]
                           and jobs[ji][1] < NCH - 1):
                        xg1_job(*jobs[ji]); ji += 1; budget -= 1

                # drain remaining nch <= 2 jobs before layer-1 starts
                while ji < len(jobs) and jobs[ji][1] < NCH - 1:
                    xg1_job(*jobs[ji]); ji += 1

                # layer-1 scan; the last xg1 chunk (needed from step 19)
                # fills the gaps of steps 0..8.
                h2prev = zstate
                for t in range(SEQ_USED):
                    h2new = work.tile([128, KH, B], f16, tag="h2", bufs=2)
                    gru_step(t, whh1_sb, h2prev, h2new, psg)
                    h2prev = h2new
                    budget = 2
                    while budget > 0 and ji < len(jobs):
                        xg1_job(*jobs[ji]); ji += 1; budget -= 1

                # logits = h2[24].T @ wcls + bcls ; y = sigmoid(logits)
                pc = psB.tile([B, 1], mybir.dt.float32, tag="psB", name="pc")
                for k in range(KH):
                    nc.tensor.matmul(
                        pc,
                        h2prev[:, k, :],
                        wcls_sb[:, k:k + 1],
                        start=(k == 0),
                        stop=(k == KH - 1),
                    )
                y_sb = work.tile([B, 1], f32, tag="ysb")
                nc.scalar.activation(y_sb, pc, AF.Sigmoid, bias=bcls_sb)
                nc.sync.dma_start(y_d.ap(), y_sb)

    nc.compile()
    return nc


def _prep_inputs(x, W_ih0, W_hh0, b_ih0, b_hh0, W_ih1, W_hh1, b_ih1, b_hh1,
                 W_cls, b_cls):
    """Shard + relayout the full inputs into per-core in_maps."""
    x = np.asarray(x, np.float32)
    f = lambda a: np.asarray(a, np.float32)

    def kpm(w, kchunks, dt):
        # [3H, D] -> [p, k, m] with w.T reshaped: out[p, k, m] = w[m, k*128+p]
        wt = np.ascontiguousarray(f(w).T)              # [D, 3H]
        return np.ascontiguousarray(
            wt.reshape(kchunks, 128, -1).transpose(1, 0, 2)
        ).astype(dt)

    wih0 = kpm(W_ih0, KI, np.float16)
    whh0 = kpm(W_hh0, KH, np.float16)
    wih1 = kpm(W_ih1, KH, np.float16)
    whh1 = kpm(W_hh1, KH, np.float16)
    bias0 = np.ascontiguousarray(
        (f(b_ih0) + f(b_hh0)).reshape(M3, 128).T).astype(np.float32)
    bias1 = np.ascontiguousarray(
        (f(b_ih1) + f(b_hh1)).reshape(M3, 128).T).astype(np.float32)
    wcls = np.ascontiguousarray(
        f(W_cls)[0].reshape(KH, 128).T).astype(np.float16)
    bcls64 = np.full((B, 1), float(np.asarray(b_cls).reshape(-1)[0]), np.float32)

    in_maps = []
    for c in range(NCORES):
        xs = x[:SEQ_USED, c * B:(c + 1) * B, :]        # [25, 64, 512]
        xT = np.ascontiguousarray(
            xs.transpose(2, 0, 1).reshape(KI, 128, N).transpose(1, 0, 2)
        ).astype(np.float16)                            # [128, 4, 1600]
        in_maps.append({
            "xT": xT, "wih0": wih0, "whh0": whh0, "wih1": wih1,
            "whh1": whh1, "bias0": bias0, "bias1": bias1,
            "wcls": wcls, "bcls64": bcls64,
        })
    return in_maps


def kernel(**inputs) -> np.ndarray:
    from concourse.bass_utils import run_bass_kernel_spmd

    if "nc" not in _CACHE:
        _CACHE["nc"] = _build()
    nc = _CACHE["nc"]

    in_maps = _prep_inputs(**inputs)
    res = run_bass_kernel_spmd(nc, in_maps, core_ids=list(range(NCORES)))
    outs = [np.asarray(res.results[c]["y"], np.float32) for c in range(NCORES)]
    return np.concatenate(outs, axis=0)          # [512, 1] float32


if __name__ == "__main__":
    rng = np.random.default_rng(0)
    demo = {
        "x": rng.standard_normal((64, BATCH, IN_DIM), np.float32),
        "W_ih0": rng.standard_normal((3 * HID, IN_DIM), np.float32) * 0.03,
        "W_hh0": rng.standard_normal((3 * HID, HID), np.float32) * 0.03,
        "b_ih0": rng.standard_normal(3 * HID).astype(np.float32) * 0.03,
        "b_hh0": rng.standard_normal(3 * HID).astype(np.float32) * 0.03,
        "W_ih1": rng.standard_normal((3 * HID, HID), np.float32) * 0.03,
        "W_hh1": rng.standard_normal((3 * HID, HID), np.float32) * 0.03,
        "b_ih1": rng.standard_normal(3 * HID).astype(np.float32) * 0.03,
        "b_hh1": rng.standard_normal(3 * HID).astype(np.float32) * 0.03,
        "W_cls": rng.standard_normal((1, HID), np.float32) * 0.03,
        "b_cls": rng.standard_normal(1).astype(np.float32) * 0.03,
    }
    print(kernel(**demo)[:8, 0])



# revision 2
# speedup vs baseline: 1.2048x; 1.2048x over previous
"""Trainium2 Bass kernel for a 2-layer GRU network + sigmoid classifier.

Reference computation (PyTorch-style GRU, gate order r,z,n):
    h1 = GRU0(x);  h2 = GRU1(h1);  out = sigmoid(h2[24] @ W_cls.T + b_cls)

Only h2[24] is consumed, so only timesteps 0..24 of both layers are needed.

Strategy (8 NeuronCores, data-parallel over batch: 512 -> 64 per core):
  - Layout: gate/hidden dim on SBUF partitions, batch on the free dim.
  - All input-gate projections (xg) live in ONE resident SBUF fp16 buffer
    (no DRAM scratch round-trip).  Layer-1's xg overwrites layer-0's in
    5-step column chunks once the layer-0 scan has consumed them.
  - Projection jobs (xg0 = W_ih0@x + b, xg1 = W_ih1@h1 + b) are emitted as
    (row-tile, chunk) units and WOVEN into the scan rounds, filling the PE
    gaps left by each step's serial gate-math tail.  Bias-add + fp32->fp16
    downcast alternates between ScalarE (activation Identity+bias) and
    VectorE so neither engine saturates.
  - The two layers' scans are INTERLEAVED (layer 1 lags by LAG steps), so
    each layer's recurrent matmuls execute during the other layer's
    gate-math tail -> near-zero PE idle in the steady state.
  - Classifier fused at the end (matmul + sigmoid w/ bias).
"""

import numpy as np

SEQ_USED = 25          # classifier reads h2[24]
BATCH = 512
IN_DIM = 512
HID = 768
NCORES = 8
B = BATCH // NCORES    # 64 per core
N = SEQ_USED * B       # 1600 projection columns
KI = IN_DIM // 128     # 4
KH = HID // 128        # 6
M3 = 3 * HID // 128    # 18 gate row-tiles (r: 0..5, z: 6..11, n: 12..17)
NW = 5 * B             # 320-column chunks == 5 timesteps exactly
NCH = N // NW          # 5 chunks
LAG = 7                # layer-1 scan lag (steps)
BUDGET = 15            # fill jobs woven after each main step

_CACHE = {}


def _build():
    """Build the SPMD Bass program (identical on all 8 cores)."""
    import concourse.mybir as mybir
    import concourse.tile as tile
    from concourse import bacc

    f32 = mybir.dt.float32
    f16 = mybir.dt.float16
    AF = mybir.ActivationFunctionType

    nc = bacc.Bacc("TRN2", target_bir_lowering=False, debug=False)

    # ---- I/O ----
    xT_d = nc.dram_tensor("xT", [128, KI, N], f16, kind="ExternalInput")
    wih0_d = nc.dram_tensor("wih0", [128, KI, 3 * HID], f16, kind="ExternalInput")
    whh0_d = nc.dram_tensor("whh0", [128, KH, 3 * HID], f16, kind="ExternalInput")
    wih1_d = nc.dram_tensor("wih1", [128, KH, 3 * HID], f16, kind="ExternalInput")
    whh1_d = nc.dram_tensor("whh1", [128, KH, 3 * HID], f16, kind="ExternalInput")
    bias0_d = nc.dram_tensor("bias0", [128, M3], f32, kind="ExternalInput")
    bias1_d = nc.dram_tensor("bias1", [128, M3], f32, kind="ExternalInput")
    wcls_d = nc.dram_tensor("wcls", [128, KH], f16, kind="ExternalInput")
    bcls_d = nc.dram_tensor("bcls64", [B, 1], f32, kind="ExternalInput")
    y_d = nc.dram_tensor("y", [B, 1], f32, kind="ExternalOutput")

    with tile.TileContext(nc) as tc:
        with (
            tc.tile_pool(name="const", bufs=1) as cpool,
            tc.tile_pool(name="work", bufs=2) as work,
            tc.tile_pool(name="ph1", bufs=1) as ph1,
        ):
            # ---- resident constants; DMA order == first-need order ----
            wih0_sb = ph1.tile([128, KI, 3 * HID], f16)
            nc.sync.dma_start(wih0_sb, wih0_d.ap())
            xT_sb = ph1.tile([128, KI, N], f16)
            for c in range(NCH):
                nc.sync.dma_start(
                    xT_sb[:, :, c * NW:(c + 1) * NW],
                    xT_d.ap()[:, :, c * NW:(c + 1) * NW],
                )
            bias0_sb = cpool.tile([128, M3], f32)
            nc.sync.dma_start(bias0_sb, bias0_d.ap())
            whh0_sb = cpool.tile([128, KH, 3 * HID], f16)
            nc.sync.dma_start(whh0_sb, whh0_d.ap())
            wih1_sb = cpool.tile([128, KH, 3 * HID], f16)
            nc.sync.dma_start(wih1_sb, wih1_d.ap())
            whh1_sb = cpool.tile([128, KH, 3 * HID], f16)
            nc.sync.dma_start(whh1_sb, whh1_d.ap())
            bias1_sb = cpool.tile([128, M3], f32)
            nc.sync.dma_start(bias1_sb, bias1_d.ap())
            wcls_sb = cpool.tile([128, KH], f16)
            nc.sync.dma_start(wcls_sb, wcls_d.ap())
            bcls_sb = cpool.tile([B, 1], f32)
            nc.sync.dma_start(bcls_sb, bcls_d.ap())

            xg_sb = cpool.tile([128, M3, N], f16)   # shared xg0/xg1 buffer
            h1T = cpool.tile([128, KH, N], f16)     # layer-0 outputs
            zstate = cpool.tile([128, KH, B], f16)  # h(-1) == 0
            nc.vector.memset(zstate, 0.0)

            with (
                tc.tile_pool(name="psg", bufs=2, space="PSUM") as psg,
                tc.tile_pool(name="psj", bufs=2, space="PSUM") as psj,
            ):
                # ---- projection job: xg[:, m, chunk] = W[:,m] @ src + b[m]
                biaseng = [0]

                def proj_job(wsb, src, kt, bsb, m, c):
                    ps = psj.tile([128, NW], f32, tag="psj", name=f"pj{m}_{c}")
                    for k in range(kt):
                        nc.tensor.matmul(
                            ps,
                            wsb[:, k, m * 128:(m + 1) * 128],
                            src[:, k, c * NW:(c + 1) * NW],
                            start=(k == 0),
                            stop=(k == kt - 1),
                        )
                    dst = xg_sb[:, m, c * NW:(c + 1) * NW]
                    if biaseng[0] % 2 == 0:
                        nc.scalar.add(dst, ps, bsb[:, m:m + 1])
                    else:
                        nc.vector.tensor_scalar_add(dst, ps, bsb[:, m:m + 1])
                    biaseng[0] += 1

                # ---- one GRU step; gate blocks in order r, n, z so the
                # n-chain overlaps the z matmuls and only the z chain tails.
                def gru_step(t, whh_sb, hprev, hnew_out):
                    xb = t * B
                    pg = {}
                    for g in (0, 2, 1):          # r, n, z
                        p = psg.tile([128, KH, B], mybir.dt.float32,
                                     tag=f"pg{g}", name=f"pg{g}")
                        pg[g] = p
                        for i in range(KH):
                            m = g * KH + i
                            for k in range(KH):
                                nc.tensor.matmul(
                                    p[:, i, :],
                                    whh_sb[:, k, m * 128:(m + 1) * 128],
                                    hprev[:, k, :],
                                    start=(k == 0),
                                    stop=(k == KH - 1),
                                )
                        if g == 0:
                            rpre = work.tile([128, KH, B], f16, tag="rpre")
                            nc.vector.tensor_add(
                                rpre, pg[0], xg_sb[:, 0:KH, xb:xb + B])
                            r16 = work.tile([128, KH, B], f16, tag="r16")
                            nc.scalar.activation(r16, rpre, AF.Sigmoid)
                        elif g == 2:
                            rhn = work.tile([128, KH, B], f16, tag="rhn")
                            nc.vector.tensor_mul(rhn, r16, pg[2])
                            npre = work.tile([128, KH, B], f16, tag="npre")
                            nc.vector.tensor_add(
                                npre, rhn, xg_sb[:, 2 * KH:3 * KH, xb:xb + B])
                            n16 = work.tile([128, KH, B], f16, tag="n16")
                            nc.scalar.activation(n16, npre, AF.Tanh)
                        else:
                            zpre = work.tile([128, KH, B], f16, tag="zpre")
                            nc.vector.tensor_add(
                                zpre, pg[1], xg_sb[:, KH:2 * KH, xb:xb + B])
                    # tail: z = sigmoid(zpre); h' = n + z * (h - n)
                    z16 = work.tile([128, KH, B], f16, tag="z16")
                    nc.scalar.activation(z16, zpre, AF.Sigmoid)
                    d16 = work.tile([128, KH, B], f16, tag="d16")
                    nc.vector.tensor_sub(d16, hprev, n16)
                    e16 = work.tile([128, KH, B], f16, tag="e16")
                    nc.vector.tensor_mul(e16, z16, d16)
                    nc.vector.tensor_add(hnew_out, n16, e16)

                # ---- fill-job queue: ph1 chunks 1.. then xg1 chunks 0.. ----
                # fillq[i] for i <  (NCH-1)*M3          -> ph1 chunk 1+i//M3
                # fillq[i] for i >= (NCH-1)*M3          -> xg1 chunk (i-..)//M3
                fillq = [("ph1", m, c) for c in range(1, NCH) for m in range(M3)]
                fillq += [("xg1", m, c) for c in range(NCH) for m in range(M3)]
                qi = [0]
                l0_done = [-1]

                def can_run(job):
                    kind, m, c = job
                    if kind == "ph1":
                        return True
                    return l0_done[0] >= 5 * c + 4   # h1 cols for chunk c done

                def run_job(job):
                    kind, m, c = job
                    if kind == "ph1":
                        proj_job(wih0_sb, xT_sb, KI, bias0_sb, m, c)
                    else:
                        proj_job(wih1_sb, h1T, KH, bias1_sb, m, c)

                def weave(budget):
                    n = 0
                    while (n < budget and qi[0] < len(fillq)
                           and can_run(fillq[qi[0]])):
                        run_job(fillq[qi[0]]); qi[0] += 1; n += 1

                def drain_to(target):
                    while qi[0] < target:
                        assert can_run(fillq[qi[0]]), "schedule bug"
                        run_job(fillq[qi[0]]); qi[0] += 1

                # ---- phase-1 chunk 0, then the interleaved scans ----
                for m in range(M3):
                    proj_job(wih0_sb, xT_sb, KI, bias0_sb, m, 0)

                h2prev = zstate
                for r in range(SEQ_USED + LAG):
                    if r < SEQ_USED:
                        # ph1 chunks up to r//5 must be emitted before L0
                        # step r reads their columns.
                        drain_to((r // 5) * M3)
                        hprev = zstate if r == 0 else h1T[:, :, (r - 1) * B:r * B]
                        gru_step(r, whh0_sb, hprev, h1T[:, :, r * B:(r + 1) * B])
                        l0_done[0] = r
                        weave(BUDGET)
                    if r >= LAG:
                        t = r - LAG
                        # xg1 chunk t//5 must be fully emitted first.
                        drain_to((NCH - 1) * M3 + (t // 5 + 1) * M3)
                        h2new = work.tile([128, KH, B], f16, tag="h2")
                        gru_step(t, whh1_sb, h2prev, h2new)
                        h2prev = h2new
                        weave(BUDGET)

                # logits = h2[24].T @ wcls + bcls ; y = sigmoid(logits)
                pc = psj.tile([B, 1], mybir.dt.float32, tag="psj", name="pc")
                for k in range(KH):
                    nc.tensor.matmul(
                        pc,
                        h2prev[:, k, :],
                        wcls_sb[:, k:k + 1],
                        start=(k == 0),
                        stop=(k == KH - 1),
                    )
                y_sb = work.tile([B, 1], f32, tag="ysb")
                nc.scalar.activation(y_sb, pc, AF.Sigmoid, bias=bcls_sb)
                nc.sync.dma_start(y_d.ap(), y_sb)

    nc.compile()
    return nc


def _prep_inputs(x, W_ih0, W_hh0, b_ih0, b_hh0, W_ih1, W_hh1, b_ih1, b_hh1,
                 W_cls, b_cls):
    """Shard + relayout the full inputs into per-core in_maps."""
    x = np.asarray(x, np.float32)
    f = lambda a: np.asarray(a, np.float32)

    def kpm(w, kchunks, dt):
        # [3H, D] -> [p, k, m] with w.T reshaped: out[p, k, m] = w[m, k*128+p]
        wt = np.ascontiguousarray(f(w).T)              # [D, 3H]
        return np.ascontiguousarray(
            wt.reshape(kchunks, 128, -1).transpose(1, 0, 2)
        ).astype(dt)

    wih0 = kpm(W_ih0, KI, np.float16)
    whh0 = kpm(W_hh0, KH, np.float16)
    wih1 = kpm(W_ih1, KH, np.float16)
    whh1 = kpm(W_hh1, KH, np.float16)
    bias0 = np.ascontiguousarray(
        (f(b_ih0) + f(b_hh0)).reshape(M3, 128).T).astype(np.float32)
    bias1 = np.ascontiguousarray(
        (f(b_ih1) + f(b_hh1)).reshape(M3, 128).T).astype(np.float32)
    wcls = np.ascontiguousarray(
        f(W_cls)[0].reshape(KH, 128).T).astype(np.float16)
    bcls64 = np.full((B, 1), float(np.asarray(b_cls).reshape(-1)[0]), np.float32)

    in_maps = []
    for c in range(NCORES):
        xs = x[:SEQ_USED, c * B:(c + 1) * B, :]        # [25, 64, 512]
        xT = np.ascontiguousarray(
            xs.transpose(2, 0, 1).reshape(KI, 128, N).transpose(1, 0, 2)
        ).astype(np.float16)                            # [128, 4, 1600]
        in_maps.append({
            "xT": xT, "wih0": wih0, "whh0": whh0, "wih1": wih1,
            "whh1": whh1, "bias0": bias0, "bias1": bias1,
            "wcls": wcls, "bcls64": bcls64,
        })
    return in_maps


def kernel(**inputs) -> np.ndarray:
    from concourse.bass_utils import run_bass_kernel_spmd

    if "nc" not in _CACHE:
        _CACHE["nc"] = _build()
    nc = _CACHE["nc"]

    in_maps = _prep_inputs(**inputs)
    res = run_bass_kernel_spmd(nc, in_maps, core_ids=list(range(NCORES)))
    outs = [np.asarray(res.results[c]["y"], np.float32) for c in range(NCORES)]
    return np.concatenate(outs, axis=0)          # [512, 1] float32


if __name__ == "__main__":
    rng = np.random.default_rng(0)
    demo = {
        "x": rng.standard_normal((64, BATCH, IN_DIM), np.float32),
        "W_ih0": rng.standard_normal((3 * HID, IN_DIM), np.float32) * 0.03,
        "W_hh0": rng.standard_normal((3 * HID, HID), np.float32) * 0.03,
        "b_ih0": rng.standard_normal(3 * HID).astype(np.float32) * 0.03,
        "b_hh0": rng.standard_normal(3 * HID).astype(np.float32) * 0.03,
        "W_ih1": rng.standard_normal((3 * HID, HID), np.float32) * 0.03,
        "W_hh1": rng.standard_normal((3 * HID, HID), np.float32) * 0.03,
        "b_ih1": rng.standard_normal(3 * HID).astype(np.float32) * 0.03,
        "b_hh1": rng.standard_normal(3 * HID).astype(np.float32) * 0.03,
        "W_cls": rng.standard_normal((1, HID), np.float32) * 0.03,
        "b_cls": rng.standard_normal(1).astype(np.float32) * 0.03,
    }
    print(kernel(**demo)[:8, 0])


# revision 6
# speedup vs baseline: 1.5140x; 1.2567x over previous
"""Trainium2 Bass kernel for a 2-layer GRU network + sigmoid classifier.

Reference computation (PyTorch-style GRU, gate order r,z,n):
    h1 = GRU0(x);  h2 = GRU1(h1);  out = sigmoid(h2[24] @ W_cls.T + b_cls)

Only h2[24] is consumed, so only timesteps 0..24 of both layers are needed.

Strategy (8 NeuronCores, data-parallel over batch: 512 -> 64 per core):
  - Layout: gate/hidden dim on SBUF partitions, batch on the free dim.
  - All input-gate projections (xg) live in ONE resident SBUF fp16 buffer
    (no DRAM scratch round-trip).  Layer-1's xg overwrites layer-0's in
    5-step column chunks once the layer-0 scan has consumed them.
  - Projection jobs (xg0 = W_ih0@x + b, xg1 = W_ih1@h1 + b) are emitted as
    (row-tile, chunk) units and WOVEN into the scan rounds, filling the PE
    gaps left by each step's serial gate-math tail.  Bias-add + fp32->fp16
    downcast alternates between ScalarE (activation Identity+bias) and
    VectorE so neither engine saturates.
  - The two layers' scans are INTERLEAVED (layer 1 lags by LAG steps), so
    each layer's recurrent matmuls execute during the other layer's
    gate-math tail -> near-zero PE idle in the steady state.
  - Classifier fused at the end (matmul + sigmoid w/ bias).
"""

import numpy as np

SEQ_USED = 25          # classifier reads h2[24]
BATCH = 512
IN_DIM = 512
HID = 768
NCORES = 8
B = BATCH // NCORES    # 64 per core
N = SEQ_USED * B       # 1600 projection columns
KI = IN_DIM // 128     # 4
KH = HID // 128        # 6
M3 = 3 * HID // 128    # 18 gate row-tiles (r: 0..5, z: 6..11, n: 12..17)
NW = 5 * B             # 320-column chunks == 5 timesteps exactly
NCH = N // NW          # 5 chunks
LAG = 5                # layer-1 scan lag (steps)
BUDGET = 15            # fill jobs woven after each main step
NWARM = 110            # dummy matmuls to lift the HAM clock gate at start

_CACHE = {}


def _build():
    """Build the SPMD Bass program (identical on all 8 cores)."""
    import concourse.mybir as mybir
    import concourse.tile as tile
    from concourse import bacc

    f32 = mybir.dt.float32
    f16 = mybir.dt.float16
    AF = mybir.ActivationFunctionType

    nc = bacc.Bacc("TRN2", target_bir_lowering=False, debug=False)

    # ---- I/O ----
    xT_d = nc.dram_tensor("xT", [128, KI, N], f16, kind="ExternalInput")
    wih0_d = nc.dram_tensor("wih0", [128, KI, 3 * HID], f16, kind="ExternalInput")
    whh0_d = nc.dram_tensor("whh0", [128, KH, 3 * HID], f16, kind="ExternalInput")
    wih1_d = nc.dram_tensor("wih1", [128, KH, 3 * HID], f16, kind="ExternalInput")
    whh1_d = nc.dram_tensor("whh1", [128, KH, 3 * HID], f16, kind="ExternalInput")
    bias0_d = nc.dram_tensor("bias0", [128, M3], f32, kind="ExternalInput")
    bias1_d = nc.dram_tensor("bias1", [128, M3], f32, kind="ExternalInput")
    wcls_d = nc.dram_tensor("wcls", [128, KH], f16, kind="ExternalInput")
    bcls_d = nc.dram_tensor("bcls64", [B, 1], f32, kind="ExternalInput")
    y_d = nc.dram_tensor("y", [B, 1], f32, kind="ExternalOutput")

    with tile.TileContext(nc) as tc:
        with (
            tc.tile_pool(name="const", bufs=1) as cpool,
            tc.tile_pool(name="work", bufs=2) as work,
            tc.tile_pool(name="ph1", bufs=1) as ph1,
        ):
            # ---- resident constants; DMA order == first-need order ----
            wih0_sb = ph1.tile([128, KI, 3 * HID], f16)
            nc.sync.dma_start(wih0_sb, wih0_d.ap())
            xT_sb = ph1.tile([128, KI, N], f16)
            nc.sync.dma_start(xT_sb, xT_d.ap())
            bias0_sb = cpool.tile([128, M3], f32)
            nc.sync.dma_start(bias0_sb, bias0_d.ap())
            whh0_sb = cpool.tile([128, KH, 3 * HID], f16)
            nc.sync.dma_start(whh0_sb, whh0_d.ap())
            wih1_sb = cpool.tile([128, KH, 3 * HID], f16)
            nc.sync.dma_start(wih1_sb, wih1_d.ap())
            whh1_sb = cpool.tile([128, KH, 3 * HID], f16)
            nc.sync.dma_start(whh1_sb, whh1_d.ap())
            bias1_sb = cpool.tile([128, M3], f32)
            nc.sync.dma_start(bias1_sb, bias1_d.ap())
            wcls_sb = cpool.tile([128, KH], f16)
            nc.sync.dma_start(wcls_sb, wcls_d.ap())
            bcls_sb = cpool.tile([B, 1], f32)
            nc.sync.dma_start(bcls_sb, bcls_d.ap())

            xg_sb = cpool.tile([128, M3, N], f16)   # shared xg0/xg1 buffer
            h1T = cpool.tile([128, KH, N], f16)     # layer-0 outputs
            zstate = cpool.tile([128, KH, B], f16)  # h(-1) == 0
            nc.vector.memset(zstate, 0.0)

            with (
                tc.tile_pool(name="psg", bufs=1, space="PSUM") as psg,
                tc.tile_pool(name="psj", bufs=4, space="PSUM") as psj,
                tc.tile_pool(name="pswm", bufs=1, space="PSUM") as pswm,
            ):
                # Warm-up while input DMAs are in flight: dummy matmuls lift
                # the PE HAM clock gate to 8/8, and a tiny sigmoid pulls in
                # the ACT table load, so phase 1 starts at full speed.
                warm_ps = pswm.tile([B, B], mybir.dt.float32, tag="warm")
                for w in range(NWARM):
                    nc.tensor.matmul(warm_ps, zstate[:, 0, :], zstate[:, 0, :],
                                     start=True, stop=True)
                warm_sg = work.tile([B, 1], f16, tag="warmsg")
                nc.scalar.activation(warm_sg, warm_ps[:, 0:1], AF.Sigmoid)
                # ---- projection job: xg[:, m, chunk] = W[:,m] @ src + b[m]
                biaseng = [0]

                def proj_job(wsb, src, kt, bsb, m, c):
                    ps = psj.tile([128, NW], f32, tag="psj", name=f"pj{m}_{c}")
                    for k in range(kt):
                        nc.tensor.matmul(
                            ps,
                            wsb[:, k, m * 128:(m + 1) * 128],
                            src[:, k, c * NW:(c + 1) * NW],
                            start=(k == 0),
                            stop=(k == kt - 1),
                        )
                    dst = xg_sb[:, m, c * NW:(c + 1) * NW]
                    if biaseng[0] % 2 == 0:
                        nc.scalar.add(dst, ps, bsb[:, m:m + 1])
                    else:
                        nc.vector.tensor_scalar_add(dst, ps, bsb[:, m:m + 1])
                    biaseng[0] += 1

                # ---- one GRU step; gate blocks in order r, n, z so the
                # n-chain overlaps the z matmuls and only the z chain tails.
                def gru_step(t, whh_sb, hprev, hnew_out):
                    xb = t * B
                    pg = {}
                    for g in (0, 2, 1):          # r, n, z
                        p = psg.tile([128, KH, B], mybir.dt.float32,
                                     tag=f"pg{g}", name=f"pg{g}")
                        pg[g] = p
                        for i in range(KH):
                            m = g * KH + i
                            for k in range(KH):
                                nc.tensor.matmul(
                                    p[:, i, :],
                                    whh_sb[:, k, m * 128:(m + 1) * 128],
                                    hprev[:, k, :],
                                    start=(k == 0),
                                    stop=(k == KH - 1),
                                )
                        if g == 0:
                            rpre = work.tile([128, KH, B], f16, tag="rpre")
                            nc.vector.tensor_add(
                                rpre, pg[0], xg_sb[:, 0:KH, xb:xb + B])
                            r16 = work.tile([128, KH, B], f16, tag="r16")
                            nc.scalar.activation(r16, rpre, AF.Sigmoid)
                        elif g == 2:
                            rhn = work.tile([128, KH, B], f16, tag="rhn")
                            nc.vector.tensor_mul(rhn, r16, pg[2])
                            npre = work.tile([128, KH, B], f16, tag="npre")
                            nc.vector.tensor_add(
                                npre, rhn, xg_sb[:, 2 * KH:3 * KH, xb:xb + B])
                            n16 = work.tile([128, KH, B], f16, tag="n16")
                            nc.scalar.activation(n16, npre, AF.Tanh)
                        else:
                            zpre = work.tile([128, KH, B], f16, tag="zpre")
                            nc.vector.tensor_add(
                                zpre, pg[1], xg_sb[:, KH:2 * KH, xb:xb + B])
                    # tail: z = sigmoid(zpre); h' = n + z * (h - n)
                    z16 = work.tile([128, KH, B], f16, tag="z16")
                    nc.scalar.activation(z16, zpre, AF.Sigmoid)
                    d16 = work.tile([128, KH, B], f16, tag="d16")
                    nc.vector.tensor_sub(d16, hprev, n16)
                    e16 = work.tile([128, KH, B], f16, tag="e16")
                    nc.vector.tensor_mul(e16, z16, d16)
                    nc.vector.tensor_add(hnew_out, n16, e16)

                # ---- fill-job queue: ph1 chunks 1.. then xg1 chunks 0.. ----
                # fillq[i] for i <  (NCH-1)*M3          -> ph1 chunk 1+i//M3
                # fillq[i] for i >= (NCH-1)*M3          -> xg1 chunk (i-..)//M3
                fillq = [("ph1", m, c) for c in range(1, NCH) for m in range(M3)]
                fillq += [("xg1", m, c) for c in range(NCH) for m in range(M3)]
                qi = [0]
                l0_done = [-1]

                def can_run(job):
                    kind, m, c = job
                    if kind == "ph1":
                        return True
                    return l0_done[0] >= 5 * c + 4   # h1 cols for chunk c done

                def run_job(job):
                    kind, m, c = job
                    if kind == "ph1":
                        proj_job(wih0_sb, xT_sb, KI, bias0_sb, m, c)
                    else:
                        proj_job(wih1_sb, h1T, KH, bias1_sb, m, c)

                def weave(budget):
                    n = 0
                    while (n < budget and qi[0] < len(fillq)
                           and can_run(fillq[qi[0]])):
                        run_job(fillq[qi[0]]); qi[0] += 1; n += 1

                def drain_to(target):
                    while qi[0] < target:
                        assert can_run(fillq[qi[0]]), "schedule bug"
                        run_job(fillq[qi[0]]); qi[0] += 1

                # ---- phase-1 chunk 0, then the interleaved scans ----
                for m in range(M3):
                    proj_job(wih0_sb, xT_sb, KI, bias0_sb, m, 0)

                h2prev = zstate
                for r in range(SEQ_USED + LAG):
                    if r < SEQ_USED:
                        # ph1 chunks up to r//5 must be emitted before L0
                        # step r reads their columns.
                        drain_to((r // 5) * M3)
                        hprev = zstate if r == 0 else h1T[:, :, (r - 1) * B:r * B]
                        gru_step(r, whh0_sb, hprev, h1T[:, :, r * B:(r + 1) * B])
                        l0_done[0] = r
                        weave(BUDGET)
                    if r >= LAG:
                        t = r - LAG
                        # xg1 chunk t//5 must be fully emitted first.
                        drain_to((NCH - 1) * M3 + (t // 5 + 1) * M3)
                        h2new = work.tile([128, KH, B], f16, tag="h2")
                        gru_step(t, whh1_sb, h2prev, h2new)
                        h2prev = h2new
                        weave(BUDGET)

                # logits = h2[24].T @ wcls + bcls ; y = sigmoid(logits)
                pc = psj.tile([B, 1], mybir.dt.float32, tag="psj", name="pc")
                for k in range(KH):
                    nc.tensor.matmul(
                        pc,
                        h2prev[:, k, :],
                        wcls_sb[:, k:k + 1],
                        start=(k == 0),
                        stop=(k == KH - 1),
                    )
                y_sb = work.tile([B, 1], f32, tag="ysb")
                nc.scalar.activation(y_sb, pc, AF.Sigmoid, bias=bcls_sb)
                nc.sync.dma_start(y_d.ap(), y_sb)

    nc.compile()
    return nc


def _prep_inputs(x, W_ih0, W_hh0, b_ih0, b_hh0, W_ih1, W_hh1, b_ih1, b_hh1,
                 W_cls, b_cls):
    """Shard + relayout the full inputs into per-core in_maps."""
    x = np.asarray(x, np.float32)
    f = lambda a: np.asarray(a, np.float32)

    def kpm(w, kchunks, dt):
        # [3H, D] -> [p, k, m] with w.T reshaped: out[p, k, m] = w[m, k*128+p]
        wt = np.ascontiguousarray(f(w).T)              # [D, 3H]
        return np.ascontiguousarray(
            wt.reshape(kchunks, 128, -1).transpose(1, 0, 2)
        ).astype(dt)

    wih0 = kpm(W_ih0, KI, np.float16)
    whh0 = kpm(W_hh0, KH, np.float16)
    wih1 = kpm(W_ih1, KH, np.float16)
    whh1 = kpm(W_hh1, KH, np.float16)
    bias0 = np.ascontiguousarray(
        (f(b_ih0) + f(b_hh0)).reshape(M3, 128).T).astype(np.float32)
    bias1 = np.ascontiguousarray(
        (f(b_ih1) + f(b_hh1)).reshape(M3, 128).T).astype(np.float32)
    wcls = np.ascontiguousarray(
        f(W_cls)[0].reshape(KH, 128).T).astype(np.float16)
    bcls64 = np.full((B, 1), float(np.asarray(b_cls).reshape(-1)[0]), np.float32)

    in_maps = []
    for c in range(NCORES):
        xs = x[:SEQ_USED, c * B:(c + 1) * B, :]        # [25, 64, 512]
        xT = np.ascontiguousarray(
            xs.transpose(2, 0, 1).reshape(KI, 128, N).transpose(1, 0, 2)
        ).astype(np.float16)                            # [128, 4, 1600]
        in_maps.append({
            "xT": xT, "wih0": wih0, "whh0": whh0, "wih1": wih1,
            "whh1": whh1, "bias0": bias0, "bias1": bias1,
            "wcls": wcls, "bcls64": bcls64,
        })
    return in_maps


def kernel(**inputs) -> np.ndarray:
    from concourse.bass_utils import run_bass_kernel_spmd

    if "nc" not in _CACHE:
        _CACHE["nc"] = _build()
    nc = _CACHE["nc"]

    in_maps = _prep_inputs(**inputs)
    res = run_bass_kernel_spmd(nc, in_maps, core_ids=list(range(NCORES)))
    outs = [np.asarray(res.results[c]["y"], np.float32) for c in range(NCORES)]
    return np.concatenate(outs, axis=0)          # [512, 1] float32


if __name__ == "__main__":
    rng = np.random.default_rng(0)
    demo = {
        "x": rng.standard_normal((64, BATCH, IN_DIM), np.float32),
        "W_ih0": rng.standard_normal((3 * HID, IN_DIM), np.float32) * 0.03,
        "W_hh0": rng.standard_normal((3 * HID, HID), np.float32) * 0.03,
        "b_ih0": rng.standard_normal(3 * HID).astype(np.float32) * 0.03,
        "b_hh0": rng.standard_normal(3 * HID).astype(np.float32) * 0.03,
        "W_ih1": rng.standard_normal((3 * HID, HID), np.float32) * 0.03,
        "W_hh1": rng.standard_normal((3 * HID, HID), np.float32) * 0.03,
        "b_ih1": rng.standard_normal(3 * HID).astype(np.float32) * 0.03,
        "b_hh1": rng.standard_normal(3 * HID).astype(np.float32) * 0.03,
        "W_cls": rng.standard_normal((1, HID), np.float32) * 0.03,
        "b_cls": rng.standard_normal(1).astype(np.float32) * 0.03,
    }
    print(kernel(**demo)[:8, 0])


# revision 11
# speedup vs baseline: 1.5308x; 1.0110x over previous
"""Trainium2 Bass kernel for a 2-layer GRU network + sigmoid classifier.

Reference computation (PyTorch-style GRU, gate order r,z,n):
    h1 = GRU0(x);  h2 = GRU1(h1);  out = sigmoid(h2[24] @ W_cls.T + b_cls)

Only h2[24] is consumed, so only timesteps 0..24 of both layers are needed.

Strategy (8 NeuronCores, data-parallel over batch: 512 -> 64 per core):
  - Layout: gate/hidden dim on SBUF partitions, batch on the free dim.
  - All input-gate projections (xg) live in ONE resident SBUF fp16 buffer
    (no DRAM scratch round-trip).  Layer-1's xg overwrites layer-0's in
    5-step column chunks once the layer-0 scan has consumed them.
  - Projection jobs (xg0 = W_ih0@x + b, xg1 = W_ih1@h1 + b) are emitted as
    (row-tile, chunk) units and WOVEN into the scan rounds, filling the PE
    gaps left by each step's serial gate-math tail.  Bias-add + fp32->fp16
    downcast alternates between ScalarE (activation Identity+bias) and
    VectorE so neither engine saturates.
  - The two layers' scans are INTERLEAVED (layer 1 lags by LAG steps), so
    each layer's recurrent matmuls execute during the other layer's
    gate-math tail -> near-zero PE idle in the steady state.
  - Classifier fused at the end (matmul + sigmoid w/ bias).
"""

import numpy as np

SEQ_USED = 25          # classifier reads h2[24]
BATCH = 512
IN_DIM = 512
HID = 768
NCORES = 8
B = BATCH // NCORES    # 64 per core
N = SEQ_USED * B       # 1600 projection columns
KI = IN_DIM // 128     # 4
KH = HID // 128        # 6
M3 = 3 * HID // 128    # 18 gate row-tiles (r: 0..5, z: 6..11, n: 12..17)
NW = 5 * B             # 320-column chunks == 5 timesteps exactly
NCH = N // NW          # 5 chunks
LAG = 3                # layer-1 scan lag (steps)
BUDGET = 15            # fill jobs woven after each main step
NWARM = 230            # dummy matmuls bridging the input-DMA lead-in
XG1R = 3               # earliest round to emit xg1 jobs (wih1 DMA landed)

_CACHE = {}


def _build():
    """Build the SPMD Bass program (identical on all 8 cores)."""
    import concourse.mybir as mybir
    import concourse.tile as tile
    from concourse import bacc

    f32 = mybir.dt.float32
    f16 = mybir.dt.float16
    AF = mybir.ActivationFunctionType

    nc = bacc.Bacc("TRN2", target_bir_lowering=False, debug=False)

    # ---- I/O ----
    xT_d = nc.dram_tensor("xT", [128, KI, N], f16, kind="ExternalInput")
    wih0_d = nc.dram_tensor("wih0", [128, KI, 3 * HID], f16, kind="ExternalInput")
    whh0_d = nc.dram_tensor("whh0", [128, KH, 3 * HID], f16, kind="ExternalInput")
    wih1_d = nc.dram_tensor("wih1", [128, KH, 3 * HID], f16, kind="ExternalInput")
    whh1_d = nc.dram_tensor("whh1", [128, KH, 3 * HID], f16, kind="ExternalInput")
    bias0_d = nc.dram_tensor("bias0", [128, M3], f32, kind="ExternalInput")
    bias1_d = nc.dram_tensor("bias1", [128, M3], f32, kind="ExternalInput")
    wcls_d = nc.dram_tensor("wcls", [128, KH], f16, kind="ExternalInput")
    bcls_d = nc.dram_tensor("bcls64", [B, 1], f32, kind="ExternalInput")
    y_d = nc.dram_tensor("y", [B, 1], f32, kind="ExternalOutput")

    with tile.TileContext(nc) as tc:
        with (
            tc.tile_pool(name="const", bufs=1) as cpool,
            tc.tile_pool(name="work", bufs=2) as work,
            tc.tile_pool(name="ph1", bufs=1) as ph1,
        ):
            # ---- resident constants; DMA order == first-need order ----
            # Big weights serial on the sync queue in first-need order (so
            # the early ones get full HBM bandwidth); small biases on the
            # scalar engine's queue so they land immediately.
            wih0_sb = ph1.tile([128, KI, 3 * HID], f16)
            nc.sync.dma_start(wih0_sb, wih0_d.ap())
            xT_sb = ph1.tile([128, KI, N], f16)
            nc.sync.dma_start(xT_sb, xT_d.ap())
            whh0_sb = cpool.tile([128, KH, 3 * HID], f16)
            nc.sync.dma_start(whh0_sb, whh0_d.ap())
            whh1_sb = cpool.tile([128, KH, 3 * HID], f16)
            nc.sync.dma_start(whh1_sb, whh1_d.ap())
            wih1_sb = cpool.tile([128, KH, 3 * HID], f16)
            nc.sync.dma_start(wih1_sb, wih1_d.ap())
            bias0_sb = cpool.tile([128, M3], f32)
            nc.scalar.dma_start(bias0_sb, bias0_d.ap())
            bias1_sb = cpool.tile([128, M3], f32)
            nc.scalar.dma_start(bias1_sb, bias1_d.ap())
            wcls_sb = cpool.tile([128, KH], f16)
            nc.scalar.dma_start(wcls_sb, wcls_d.ap())
            bcls_sb = cpool.tile([B, 1], f32)
            nc.scalar.dma_start(bcls_sb, bcls_d.ap())

            xg_sb = cpool.tile([128, M3, N], f16)   # shared xg0/xg1 buffer
            h1T = cpool.tile([128, KH, N], f16)     # layer-0 outputs
            zstate = cpool.tile([128, KH, B], f16)  # h(-1) == 0
            nc.vector.memset(zstate, 0.0)

            with (
                tc.tile_pool(name="psg", bufs=1, space="PSUM") as psg,
                tc.tile_pool(name="psj", bufs=4, space="PSUM") as psj,
                tc.tile_pool(name="pswm", bufs=1, space="PSUM") as pswm,
            ):
                # Warm-up while input DMAs are in flight: dummy matmuls lift
                # the PE HAM clock gate to 8/8, and a tiny sigmoid pulls in
                # the ACT table load, so phase 1 starts at full speed.
                warm_ps = pswm.tile([B, B], mybir.dt.float32, tag="warm")
                for w in range(NWARM):
                    nc.tensor.matmul(warm_ps, zstate[:, 0, :], zstate[:, 0, :],
                                     start=True, stop=True)
                warm_sg = work.tile([B, 1], f16, tag="warmsg")
                nc.scalar.activation(warm_sg, warm_ps[:, 0:1], AF.Sigmoid)
                # ---- projection job: xg[:, m, lo:hi] = W[:,m] @ src + b[m]
                biaseng = [0]

                def proj_job(wsb, src, kt, bsb, m, lo, hi):
                    ps = psj.tile([128, NW], f32, tag="psj", name=f"pj{m}_{lo}")
                    w = hi - lo
                    for k in range(kt):
                        nc.tensor.matmul(
                            ps[:, 0:w],
                            wsb[:, k, m * 128:(m + 1) * 128],
                            src[:, k, lo:hi],
                            start=(k == 0),
                            stop=(k == kt - 1),
                        )
                    dst = xg_sb[:, m, lo:hi]
                    if biaseng[0] % 2 == 0:
                        nc.scalar.add(dst, ps[:, 0:w], bsb[:, m:m + 1])
                    else:
                        nc.vector.tensor_scalar_add(dst, ps[:, 0:w], bsb[:, m:m + 1])
                    biaseng[0] += 1

                # ---- one GRU step; gate blocks in order r, n, z so the
                # n-chain overlaps the z matmuls and only the z chain tails.
                def gru_step(t, whh_sb, hprev, hnew_out):
                    xb = t * B
                    pg = {}
                    for g in (0, 2, 1):          # r, n, z
                        p = psg.tile([128, KH, B], mybir.dt.float32,
                                     tag=f"pg{g}", name=f"pg{g}")
                        pg[g] = p
                        for i in range(KH):
                            m = g * KH + i
                            for k in range(KH):
                                nc.tensor.matmul(
                                    p[:, i, :],
                                    whh_sb[:, k, m * 128:(m + 1) * 128],
                                    hprev[:, k, :],
                                    start=(k == 0),
                                    stop=(k == KH - 1),
                                )
                        if g == 0:
                            rpre = work.tile([128, KH, B], f16, tag="rpre")
                            nc.vector.tensor_add(
                                rpre, pg[0], xg_sb[:, 0:KH, xb:xb + B])
                            r16 = work.tile([128, KH, B], f16, tag="r16")
                            nc.scalar.activation(r16, rpre, AF.Sigmoid)
                        elif g == 2:
                            rhn = work.tile([128, KH, B], f16, tag="rhn")
                            nc.vector.tensor_mul(rhn, r16, pg[2])
                            npre = work.tile([128, KH, B], f16, tag="npre")
                            nc.vector.tensor_add(
                                npre, rhn, xg_sb[:, 2 * KH:3 * KH, xb:xb + B])
                            n16 = work.tile([128, KH, B], f16, tag="n16")
                            nc.scalar.activation(n16, npre, AF.Tanh)
                        else:
                            zpre = work.tile([128, KH, B], f16, tag="zpre")
                            nc.vector.tensor_add(
                                zpre, pg[1], xg_sb[:, KH:2 * KH, xb:xb + B])
                    # tail: z = sigmoid(zpre); h' = n + z * (h - n)
                    z16 = work.tile([128, KH, B], f16, tag="z16")
                    nc.scalar.activation(z16, zpre, AF.Sigmoid)
                    d16 = work.tile([128, KH, B], f16, tag="d16")
                    nc.vector.tensor_sub(d16, hprev, n16)
                    e16 = work.tile([128, KH, B], f16, tag="e16")
                    nc.vector.tensor_mul(e16, z16, d16)
                    nc.vector.tensor_add(hnew_out, n16, e16)

                # ---- fill-job queue ----
                # job = (kind, m, lo, hi, ready_step). xg1 jobs additionally
                # wait for round >= XG1R so their matmuls never sit in the
                # in-order PE queue blocked on the wih1/whh1 DMAs.
                # xg1 chunks split into 2-step + 3-step spans: LAG must be
                # >= the span length in steps, so spans <= 3 steps => LAG=3.
                xg1_spans = []
                for c in range(NCH):
                    xg1_spans.append((c * NW, c * NW + 2 * B, 5 * c + 1))
                    xg1_spans.append((c * NW + 2 * B, (c + 1) * NW, 5 * c + 4))
                fillq = [("ph1", m, c * NW, (c + 1) * NW, -1)
                         for c in range(1, NCH) for m in range(M3)]
                fillq += [("xg1", m, lo, hi, rdy)
                          for (lo, hi, rdy) in xg1_spans for m in range(M3)]
                qi = [0]
                l0_done = [-1]
                rnd = [0]

                def can_run(job):
                    kind, m, lo, hi, rdy = job
                    if kind == "ph1":
                        return True
                    return l0_done[0] >= rdy and rnd[0] >= XG1R

                def run_job(job):
                    kind, m, lo, hi, rdy = job
                    if kind == "ph1":
                        proj_job(wih0_sb, xT_sb, KI, bias0_sb, m, lo, hi)
                    else:
                        proj_job(wih1_sb, h1T, KH, bias1_sb, m, lo, hi)

                def weave(budget):
                    n = 0
                    while (n < budget and qi[0] < len(fillq)
                           and can_run(fillq[qi[0]])):
                        run_job(fillq[qi[0]]); qi[0] += 1; n += 1

                def drain_cols(kind, col):
                    # emit all `kind` jobs whose span starts below `col`
                    while qi[0] < len(fillq):
                        k2, m, lo, hi, rdy = fillq[qi[0]]
                        if k2 != kind and kind == "ph1":
                            break               # ph1 jobs are all up front
                        if k2 == kind and lo >= col:
                            break
                        assert can_run(fillq[qi[0]]), "schedule bug"
                        run_job(fillq[qi[0]]); qi[0] += 1

                # ---- phase-1 chunk 0, then the interleaved scans ----
                for m in range(M3):
                    proj_job(wih0_sb, xT_sb, KI, bias0_sb, m, 0, NW)

                h2prev = zstate
                for r in range(SEQ_USED + LAG):
                    rnd[0] = r
                    if r < SEQ_USED:
                        drain_cols("ph1", (r + 1) * B)
                        hprev = zstate if r == 0 else h1T[:, :, (r - 1) * B:r * B]
                        gru_step(r, whh0_sb, hprev, h1T[:, :, r * B:(r + 1) * B])
                        l0_done[0] = r
                        weave(BUDGET)
                    if r >= LAG:
                        t = r - LAG
                        drain_cols("xg1", (t + 1) * B)
                        h2new = work.tile([128, KH, B], f16, tag="h2")
                        gru_step(t, whh1_sb, h2prev, h2new)
                        h2prev = h2new
                        weave(BUDGET)

                # logits = h2[24].T @ wcls + bcls ; y = sigmoid(logits)
                pc = psj.tile([B, 1], mybir.dt.float32, tag="psj", name="pc")
                for k in range(KH):
                    nc.tensor.matmul(
                        pc,
                        h2prev[:, k, :],
                        wcls_sb[:, k:k + 1],
                        start=(k == 0),
                        stop=(k == KH - 1),
                    )
                y_sb = work.tile([B, 1], f32, tag="ysb")
                nc.scalar.activation(y_sb, pc, AF.Sigmoid, bias=bcls_sb)
                nc.sync.dma_start(y_d.ap(), y_sb)

    nc.compile()
    return nc


def _prep_inputs(x, W_ih0, W_hh0, b_ih0, b_hh0, W_ih1, W_hh1, b_ih1, b_hh1,
                 W_cls, b_cls):
    """Shard + relayout the full inputs into per-core in_maps."""
    x = np.asarray(x, np.float32)
    f = lambda a: np.asarray(a, np.float32)

    def kpm(w, kchunks, dt):
        # [3H, D] -> [p, k, m] with w.T reshaped: out[p, k, m] = w[m, k*128+p]
        wt = np.ascontiguousarray(f(w).T)              # [D, 3H]
        return np.ascontiguousarray(
            wt.reshape(kchunks, 128, -1).transpose(1, 0, 2)
        ).astype(dt)

    wih0 = kpm(W_ih0, KI, np.float16)
    whh0 = kpm(W_hh0, KH, np.float16)
    wih1 = kpm(W_ih1, KH, np.float16)
    whh1 = kpm(W_hh1, KH, np.float16)
    bias0 = np.ascontiguousarray(
        (f(b_ih0) + f(b_hh0)).reshape(M3, 128).T).astype(np.float32)
    bias1 = np.ascontiguousarray(
        (f(b_ih1) + f(b_hh1)).reshape(M3, 128).T).astype(np.float32)
    wcls = np.ascontiguousarray(
        f(W_cls)[0].reshape(KH, 128).T).astype(np.float16)
    bcls64 = np.full((B, 1), float(np.asarray(b_cls).reshape(-1)[0]), np.float32)

    in_maps = []
    for c in range(NCORES):
        xs = x[:SEQ_USED, c * B:(c + 1) * B, :]        # [25, 64, 512]
        xT = np.ascontiguousarray(
            xs.transpose(2, 0, 1).reshape(KI, 128, N).transpose(1, 0, 2)
        ).astype(np.float16)                            # [128, 4, 1600]
        in_maps.append({
            "xT": xT, "wih0": wih0, "whh0": whh0, "wih1": wih1,
            "whh1": whh1, "bias0": bias0, "bias1": bias1,
            "wcls": wcls, "bcls64": bcls64,
        })
    return in_maps


def kernel(**inputs) -> np.ndarray:
    from concourse.bass_utils import run_bass_kernel_spmd

    if "nc" not in _CACHE:
        _CACHE["nc"] = _build()
    nc = _CACHE["nc"]

    in_maps = _prep_inputs(**inputs)
    res = run_bass_kernel_spmd(nc, in_maps, core_ids=list(range(NCORES)))
    outs = [np.asarray(res.results[c]["y"], np.float32) for c in range(NCORES)]
    return np.concatenate(outs, axis=0)          # [512, 1] float32


if __name__ == "__main__":
    rng = np.random.default_rng(0)
    demo = {
        "x": rng.standard_normal((64, BATCH, IN_DIM), np.float32),
        "W_ih0": rng.standard_normal((3 * HID, IN_DIM), np.float32) * 0.03,
        "W_hh0": rng.standard_normal((3 * HID, HID), np.float32) * 0.03,
        "b_ih0": rng.standard_normal(3 * HID).astype(np.float32) * 0.03,
        "b_hh0": rng.standard_normal(3 * HID).astype(np.float32) * 0.03,
        "W_ih1": rng.standard_normal((3 * HID, HID), np.float32) * 0.03,
        "W_hh1": rng.standard_normal((3 * HID, HID), np.float32) * 0.03,
        "b_ih1": rng.standard_normal(3 * HID).astype(np.float32) * 0.03,
        "b_hh1": rng.standard_normal(3 * HID).astype(np.float32) * 0.03,
        "W_cls": rng.standard_normal((1, HID), np.float32) * 0.03,
        "b_cls": rng.standard_normal(1).astype(np.float32) * 0.03,
    }
    print(kernel(**demo)[:8, 0])


# revision 19
# speedup vs baseline: 1.6685x; 1.0900x over previous
"""Trainium2 Bass kernel for a 2-layer GRU network + sigmoid classifier.

Reference computation (PyTorch-style GRU, gate order r,z,n):
    h1 = GRU0(x);  h2 = GRU1(h1);  out = sigmoid(h2[24] @ W_cls.T + b_cls)

Only h2[24] is consumed, so only timesteps 0..24 of both layers are needed.

Strategy (8 NeuronCores, data-parallel over batch: 512 -> 64 per core):
  - Layout: gate/hidden dim on SBUF partitions, batch on the free dim.
  - All input-gate projections (xg) live in ONE resident SBUF fp16 buffer
    (no DRAM scratch round-trip).  Layer-1's xg overwrites layer-0's in
    5-step column chunks once the layer-0 scan has consumed them.
  - Projection jobs (xg0 = W_ih0@x + b, xg1 = W_ih1@h1 + b) are emitted as
    (row-tile, chunk) units and WOVEN into the scan rounds, filling the PE
    gaps left by each step's serial gate-math tail.  Bias-add + fp32->fp16
    downcast alternates between ScalarE (activation Identity+bias) and
    VectorE so neither engine saturates.
  - The two layers' scans are INTERLEAVED (layer 1 lags by LAG steps), so
    each layer's recurrent matmuls execute during the other layer's
    gate-math tail -> near-zero PE idle in the steady state.
  - Classifier fused at the end (matmul + sigmoid w/ bias).
"""

import numpy as np

SEQ_USED = 25          # classifier reads h2[24]
BATCH = 512
IN_DIM = 512
HID = 768
NCORES = 8
B = BATCH // NCORES    # 64 per core
N = SEQ_USED * B       # 1600 projection columns
KI = IN_DIM // 128     # 4
KH = HID // 128        # 6
M3 = 3 * HID // 128    # 18 gate row-tiles (r: 0..5, z: 6..11, n: 12..17)
NW = 5 * B             # 320-column chunks == 5 timesteps exactly
NCH = N // NW          # 5 chunks
LAG = 3                # layer-1 scan lag (steps)
BUDGET = 15            # fill jobs woven after each main step
NWARM = 110            # dummy matmuls bridging the input-DMA lead-in
XG1R = 3               # earliest round to emit xg1 jobs (wih1 DMA landed)
KI2 = KI // 2          # fp8 DoubleRow pairs two 128-k-tiles per matmul
S8 = 16.0              # W_ih0 pre-scale so fp8 weights stay in normal range

_CACHE = {}


def _build():
    """Build the SPMD Bass program (identical on all 8 cores)."""
    import concourse.mybir as mybir
    import concourse.tile as tile
    from concourse import bacc

    f32 = mybir.dt.float32
    f16 = mybir.dt.float16
    f8 = mybir.dt.float8e4
    AF = mybir.ActivationFunctionType
    AOP = mybir.AluOpType

    nc = bacc.Bacc("TRN2", target_bir_lowering=False, debug=False)

    # ---- I/O ----
    xT_d = nc.dram_tensor("xT", [128, KI2, 2, N], f8, kind="ExternalInput")
    wih0_d = nc.dram_tensor("wih0", [128, KI2, 2, 3 * HID], f8, kind="ExternalInput")
    whh0_d = nc.dram_tensor("whh0", [128, KH, 3 * HID], f16, kind="ExternalInput")
    wih1_d = nc.dram_tensor("wih1", [128, KH, 3 * HID], f16, kind="ExternalInput")
    whh1_d = nc.dram_tensor("whh1", [128, KH, 3 * HID], f16, kind="ExternalInput")
    bias0_d = nc.dram_tensor("bias0", [128, M3], f32, kind="ExternalInput")
    bias1_d = nc.dram_tensor("bias1", [128, M3], f32, kind="ExternalInput")
    wcls_d = nc.dram_tensor("wcls", [128, KH], f16, kind="ExternalInput")
    bcls_d = nc.dram_tensor("bcls64", [B, 1], f32, kind="ExternalInput")
    y_d = nc.dram_tensor("y", [B, 1], f32, kind="ExternalOutput")

    with tile.TileContext(nc) as tc:
        with (
            tc.tile_pool(name="const", bufs=1) as cpool,
            tc.tile_pool(name="work", bufs=2) as work,
            tc.tile_pool(name="ph1", bufs=1) as ph1,
        ):
            # ---- resident constants; DMA order == first-need order ----
            # Big weights serial on the sync queue in first-need order (so
            # the early ones get full HBM bandwidth); small biases on the
            # scalar engine's queue so they land immediately.
            wih0_sb = ph1.tile([128, KI2, 2, 3 * HID], f8)
            nc.sync.dma_start(wih0_sb, wih0_d.ap())
            xT_sb = ph1.tile([128, KI2, 2, N], f8)
            nc.sync.dma_start(xT_sb, xT_d.ap())
            whh0_sb = cpool.tile([128, KH, 3 * HID], f16)
            nc.sync.dma_start(whh0_sb, whh0_d.ap())
            whh1_sb = cpool.tile([128, KH, 3 * HID], f16)
            nc.sync.dma_start(whh1_sb, whh1_d.ap())
            wih1_sb = cpool.tile([128, KH, 3 * HID], f16)
            nc.sync.dma_start(wih1_sb, wih1_d.ap())
            bias0_sb = cpool.tile([128, M3], f32)
            nc.scalar.dma_start(bias0_sb, bias0_d.ap())
            bias1_sb = cpool.tile([128, M3], f32)
            nc.scalar.dma_start(bias1_sb, bias1_d.ap())
            wcls_sb = cpool.tile([128, KH], f16)
            nc.scalar.dma_start(wcls_sb, wcls_d.ap())
            bcls_sb = cpool.tile([B, 1], f32)
            nc.scalar.dma_start(bcls_sb, bcls_d.ap())

            xg_sb = cpool.tile([128, M3, N], f16)   # shared xg0/xg1 buffer
            h1T = cpool.tile([128, KH, N], f16)     # layer-0 outputs
            zstate = cpool.tile([128, KH, B], f16)  # h(-1) == 0
            nc.vector.memset(zstate, 0.0)

            with (
                tc.tile_pool(name="psg", bufs=1, space="PSUM") as psg,
                tc.tile_pool(name="psj", bufs=4, space="PSUM") as psj,
                tc.tile_pool(name="pswm", bufs=1, space="PSUM") as pswm,
            ):
                # Warm-up while input DMAs are in flight: dummy matmuls lift
                # the PE HAM clock gate to 8/8, and a tiny sigmoid pulls in
                # the ACT table load, so phase 1 starts at full speed.
                warm_ps = pswm.tile([B, B], mybir.dt.float32, tag="warm")
                for w in range(NWARM):
                    nc.tensor.matmul(warm_ps, zstate[:, 0, :], zstate[:, 0, :],
                                     start=True, stop=True)
                warm_sg = work.tile([B, 1], f16, tag="warmsg")
                nc.scalar.activation(warm_sg, warm_ps[:, 0:1], AF.Sigmoid)
                # ---- projection job: xg[:, m, lo:hi] = W[:,m] @ src + b[m]
                biaseng = [0]

                def proj_job(wsb, src, kt, bsb, m, lo, hi, dr=False):
                    ps = psj.tile([128, NW], f32, tag="psj", name=f"pj{m}_{lo}")
                    w = hi - lo
                    for k in range(kt):
                        if dr:
                            # fp8 DoubleRow: (p, j) pairs of both operands
                            # contract positionally -> 256-row virtual tiles.
                            nc.tensor.matmul(
                                ps[:, 0:w],
                                wsb[:, k, :, m * 128:(m + 1) * 128],
                                src[:, k, :, lo:hi],
                                start=(k == 0),
                                stop=(k == kt - 1),
                                perf_mode=mybir.MatmulPerfMode.DoubleRow,
                            )
                        else:
                            nc.tensor.matmul(
                                ps[:, 0:w],
                                wsb[:, k, m * 128:(m + 1) * 128],
                                src[:, k, lo:hi],
                                start=(k == 0),
                                stop=(k == kt - 1),
                            )
                    dst = xg_sb[:, m, lo:hi]
                    scale = (1.0 / S8) if dr else 1.0
                    if biaseng[0] % 2 == 0:
                        nc.scalar.activation(dst, ps[:, 0:w], AF.Identity,
                                             bias=bsb[:, m:m + 1], scale=scale)
                    else:
                        if dr:
                            nc.vector.tensor_scalar(
                                dst, ps[:, 0:w], scale, bsb[:, m:m + 1],
                                op0=AOP.mult, op1=AOP.add)
                        else:
                            nc.vector.tensor_scalar_add(
                                dst, ps[:, 0:w], bsb[:, m:m + 1])
                    biaseng[0] += 1

                # ---- one GRU step; gate blocks in order r, n, z so the
                # n-chain overlaps the z matmuls and only the z chain tails.
                def gru_step(t, whh_sb, hprev, hnew_out):
                    xb = t * B
                    pg = {}
                    for g in (0, 2, 1):          # r, n, z
                        p = psg.tile([128, KH, B], mybir.dt.float32,
                                     tag=f"pg{g}", name=f"pg{g}")
                        pg[g] = p
                        for i in range(KH):
                            m = g * KH + i
                            for k in range(KH):
                                nc.tensor.matmul(
                                    p[:, i, :],
                                    whh_sb[:, k, m * 128:(m + 1) * 128],
                                    hprev[:, k, :],
                                    start=(k == 0),
                                    stop=(k == KH - 1),
                                )
                        if g == 0:
                            rpre = work.tile([128, KH, B], f16, tag="rpre")
                            nc.vector.tensor_add(
                                rpre, pg[0], xg_sb[:, 0:KH, xb:xb + B])
                            r16 = work.tile([128, KH, B], f16, tag="r16")
                            nc.scalar.activation(r16, rpre, AF.Sigmoid)
                        elif g == 2:
                            rhn = work.tile([128, KH, B], f16, tag="rhn")
                            nc.vector.tensor_mul(rhn, r16, pg[2])
                            npre = work.tile([128, KH, B], f16, tag="npre")
                            nc.vector.tensor_add(
                                npre, rhn, xg_sb[:, 2 * KH:3 * KH, xb:xb + B])
                            n16 = work.tile([128, KH, B], f16, tag="n16")
                            nc.scalar.activation(n16, npre, AF.Tanh)
                        else:
                            zpre = work.tile([128, KH, B], f16, tag="zpre")
                            nc.vector.tensor_add(
                                zpre, pg[1], xg_sb[:, KH:2 * KH, xb:xb + B])
                    # tail: z = sigmoid(zpre); h' = n + z * (h - n)
                    z16 = work.tile([128, KH, B], f16, tag="z16")
                    nc.scalar.activation(z16, zpre, AF.Sigmoid)
                    d16 = work.tile([128, KH, B], f16, tag="d16")
                    nc.vector.tensor_sub(d16, hprev, n16)
                    e16 = work.tile([128, KH, B], f16, tag="e16")
                    nc.vector.tensor_mul(e16, z16, d16)
                    nc.vector.tensor_add(hnew_out, n16, e16)

                # ---- fill-job queue ----
                # job = (kind, m, lo, hi, ready_step). xg1 jobs additionally
                # wait for round >= XG1R so their matmuls never sit in the
                # in-order PE queue blocked on the wih1/whh1 DMAs.
                # xg1 chunks split into 2-step + 3-step spans: LAG must be
                # >= the span length in steps, so spans <= 3 steps => LAG=3.
                xg1_spans = []
                for c in range(NCH):
                    xg1_spans.append((c * NW, c * NW + 2 * B, 5 * c + 1))
                    xg1_spans.append((c * NW + 2 * B, (c + 1) * NW, 5 * c + 4))
                fillq = [("ph1", m, c * NW, (c + 1) * NW, -1)
                         for c in range(1, NCH) for m in range(M3)]
                fillq += [("xg1", m, lo, hi, rdy)
                          for (lo, hi, rdy) in xg1_spans for m in range(M3)]
                qi = [0]
                l0_done = [-1]
                rnd = [0]

                def can_run(job):
                    kind, m, lo, hi, rdy = job
                    if kind == "ph1":
                        return True
                    return l0_done[0] >= rdy and rnd[0] >= XG1R

                def run_job(job):
                    kind, m, lo, hi, rdy = job
                    if kind == "ph1":
                        proj_job(wih0_sb, xT_sb, KI2, bias0_sb, m, lo, hi,
                                 dr=True)
                    else:
                        proj_job(wih1_sb, h1T, KH, bias1_sb, m, lo, hi)

                def weave(budget):
                    n = 0
                    while (n < budget and qi[0] < len(fillq)
                           and can_run(fillq[qi[0]])):
                        run_job(fillq[qi[0]]); qi[0] += 1; n += 1

                def drain_cols(kind, col):
                    # emit all `kind` jobs whose span starts below `col`
                    while qi[0] < len(fillq):
                        k2, m, lo, hi, rdy = fillq[qi[0]]
                        if k2 != kind and kind == "ph1":
                            break               # ph1 jobs are all up front
                        if k2 == kind and lo >= col:
                            break
                        assert can_run(fillq[qi[0]]), "schedule bug"
                        run_job(fillq[qi[0]]); qi[0] += 1

                # ---- phase-1 chunk 0, then the interleaved scans ----
                for m in range(M3):
                    proj_job(wih0_sb, xT_sb, KI2, bias0_sb, m, 0, NW, dr=True)

                h2prev = zstate
                for r in range(SEQ_USED + LAG):
                    rnd[0] = r
                    if r < SEQ_USED:
                        drain_cols("ph1", (r + 1) * B)
                        hprev = zstate if r == 0 else h1T[:, :, (r - 1) * B:r * B]
                        gru_step(r, whh0_sb, hprev, h1T[:, :, r * B:(r + 1) * B])
                        l0_done[0] = r
                        weave(BUDGET)
                    if r >= LAG:
                        t = r - LAG
                        drain_cols("xg1", (t + 1) * B)
                        h2new = work.tile([128, KH, B], f16, tag="h2")
                        gru_step(t, whh1_sb, h2prev, h2new)
                        h2prev = h2new
                        weave(BUDGET)

                # logits = h2[24].T @ wcls + bcls ; y = sigmoid(logits)
                pc = psj.tile([B, 1], mybir.dt.float32, tag="psj", name="pc")
                for k in range(KH):
                    nc.tensor.matmul(
                        pc,
                        h2prev[:, k, :],
                        wcls_sb[:, k:k + 1],
                        start=(k == 0),
                        stop=(k == KH - 1),
                    )
                y_sb = work.tile([B, 1], f32, tag="ysb")
                nc.scalar.activation(y_sb, pc, AF.Sigmoid, bias=bcls_sb)
                nc.sync.dma_start(y_d.ap(), y_sb)

    nc.compile()
    return nc


def _prep_inputs(x, W_ih0, W_hh0, b_ih0, b_hh0, W_ih1, W_hh1, b_ih1, b_hh1,
                 W_cls, b_cls):
    """Shard + relayout the full inputs into per-core in_maps."""
    x = np.asarray(x, np.float32)
    f = lambda a: np.asarray(a, np.float32)

    def kpm(w, kchunks, dt):
        # [3H, D] -> [p, k, m] with w.T reshaped: out[p, k, m] = w[m, k*128+p]
        wt = np.ascontiguousarray(f(w).T)              # [D, 3H]
        return np.ascontiguousarray(
            wt.reshape(kchunks, 128, -1).transpose(1, 0, 2)
        ).astype(dt)

    import ml_dtypes
    f8 = ml_dtypes.float8_e4m3
    # phase-1 operands in fp8 e4m3, laid out [128, KI2, 2, cols] so that the
    # (p, j) pairs of weight and moving operand contract positionally under
    # DoubleRow.  W_ih0 is pre-scaled by S8 to keep fp8 in the normal range;
    # the kernel rescales by 1/S8 in the bias-add.
    wih0 = kpm(W_ih0 * S8, KI, np.float32).reshape(128, KI2, 2, 3 * HID)
    wih0 = np.ascontiguousarray(wih0).astype(f8)
    whh0 = kpm(W_hh0, KH, np.float16)
    wih1 = kpm(W_ih1, KH, np.float16)
    whh1 = kpm(W_hh1, KH, np.float16)
    bias0 = np.ascontiguousarray(
        (f(b_ih0) + f(b_hh0)).reshape(M3, 128).T).astype(np.float32)
    bias1 = np.ascontiguousarray(
        (f(b_ih1) + f(b_hh1)).reshape(M3, 128).T).astype(np.float32)
    wcls = np.ascontiguousarray(
        f(W_cls)[0].reshape(KH, 128).T).astype(np.float16)
    bcls64 = np.full((B, 1), float(np.asarray(b_cls).reshape(-1)[0]), np.float32)

    in_maps = []
    for c in range(NCORES):
        xs = x[:SEQ_USED, c * B:(c + 1) * B, :]        # [25, 64, 512]
        xT = np.ascontiguousarray(
            xs.transpose(2, 0, 1).reshape(KI, 128, N).transpose(1, 0, 2)
        ).reshape(128, KI2, 2, N)                       # [128, 2, 2, 1600]
        xT = np.ascontiguousarray(xT).astype(f8)
        in_maps.append({
            "xT": xT, "wih0": wih0, "whh0": whh0, "wih1": wih1,
            "whh1": whh1, "bias0": bias0, "bias1": bias1,
            "wcls": wcls, "bcls64": bcls64,
        })
    return in_maps


def kernel(**inputs) -> np.ndarray:
    from concourse.bass_utils import run_bass_kernel_spmd

    if "nc" not in _CACHE:
        _CACHE["nc"] = _build()
    nc = _CACHE["nc"]

    in_maps = _prep_inputs(**inputs)
    res = run_bass_kernel_spmd(nc, in_maps, core_ids=list(range(NCORES)))
    outs = [np.asarray(res.results[c]["y"], np.float32) for c in range(NCORES)]
    return np.concatenate(outs, axis=0)          # [512, 1] float32


if __name__ == "__main__":
    rng = np.random.default_rng(0)
    demo = {
        "x": rng.standard_normal((64, BATCH, IN_DIM), np.float32),
        "W_ih0": rng.standard_normal((3 * HID, IN_DIM), np.float32) * 0.03,
        "W_hh0": rng.standard_normal((3 * HID, HID), np.float32) * 0.03,
        "b_ih0": rng.standard_normal(3 * HID).astype(np.float32) * 0.03,
        "b_hh0": rng.standard_normal(3 * HID).astype(np.float32) * 0.03,
        "W_ih1": rng.standard_normal((3 * HID, HID), np.float32) * 0.03,
        "W_hh1": rng.standard_normal((3 * HID, HID), np.float32) * 0.03,
        "b_ih1": rng.standard_normal(3 * HID).astype(np.float32) * 0.03,
        "b_hh1": rng.standard_normal(3 * HID).astype(np.float32) * 0.03,
        "W_cls": rng.standard_normal((1, HID), np.float32) * 0.03,
        "b_cls": rng.standard_normal(1).astype(np.float32) * 0.03,
    }
    print(kernel(**demo)[:8, 0])
